# revision 1
# baseline (speedup 1.0000x reference)
"""Trainium2 Bass kernel for the 2-layer LSTM encoder/decoder problem.

Strategy (8 NeuronCores):
  - Tensor-parallel shard of the 4L=8192 gate rows: core k owns rows
    [256k:256k+256) of each gate (i,f,g,o) -> 1024 gate rows / core.
  - Activations live transposed [feature, batch] on device; batch = 32
    (the two independent scan chains of the reference are batched).
  - Non-autoregressive phases (encoder scans, decoder consume scans) are
    processed layer-by-layer: the Wih contribution for all 4 timesteps is
    one M=128 bulk matmul (weights stream once); only the Whh recurrence
    is stepwise, with Whh SBUF-resident.
  - Matmuls run in bf16 (PSUM accumulation fp32); cell state and outputs
    stay fp32. All four decoder matrices are SBUF-resident in bf16, so
    the autoregressive phase does no weight streaming at all.
  - Hidden slices are AllGather'ed (bf16) between layer-steps; chunk
    outputs are written per-core (own slice, fp32) and gathered on host.
  - Features are globally permuted f' = 512*h + c so the final 1x1 conv
    is a plain matmul over the gathered hidden tiles.
"""

import tempfile

import numpy as np
import ml_dtypes

import concourse.bass as bass
import concourse.bacc as bacc
import concourse.mybir as mybir
import concourse.tile as tile
from concourse import bass_utils

# Problem constants (hardcoded per contract)
C, H, W = 512, 4, 4
SPLIT, PRED = 4, 4
L = 2048           # lstm feature size
B = 16             # reference batch
NB = 32            # device batch (two chains)
NCORES = 8
SL = L // NCORES   # 256: hidden slice per core
GL = 4 * SL        # 1024: gate rows per core
NT = L // 128      # 16 k-tiles
NAR = PRED + SPLIT - 1  # 7 autoregressive steps

F32 = mybir.dt.float32
BF16 = mybir.dt.bfloat16
NPBF = ml_dtypes.bfloat16

# Permutation: device feature f' = 512*h + c  <->  natural f = 4*c + h
PERM = np.array([4 * (f % C) + f // C for f in range(L)], dtype=np.int64)
IPERM = np.argsort(PERM)

_CACHE = {}


def _build_nc():
    nc = bacc.Bacc("TRN2", target_bir_lowering=False, debug=False,
                   num_devices=NCORES)

    def din(name, shape, dt=F32):
        return nc.dram_tensor(name, shape, dt, kind="ExternalInput").ap()

    def dout(name, shape):
        return nc.dram_tensor(name, shape, F32, kind="ExternalOutput").ap()

    xET = din("xET", [128, NT * 4 * NB], BF16)
    xDT = din("xDT", [128, NT * 4 * NB], BF16)
    eWih = din("eWih", [2, L, GL], BF16)
    eWhh = din("eWhh", [2, L, GL], BF16)
    dWih = din("dWih", [2, L, GL], BF16)
    dWhh = din("dWhh", [2, L, GL], BF16)
    eB = din("eB", [2, 32, GL])       # bias replicated over 32 partitions
    dB = din("dB", [2, 32, GL])
    cWT = din("cWT", [2 * C, C], BF16)      # conv_W.T
    cB = din("cB", [64, C])           # conv bias replicated over 64 rows

    # per-core own h2 slice [32 batch, 256 feat] per chunk; host gathers
    chunks_out = dout("chunks_out", [8, NB, SL])
    convout = dout("convout", [4, 64, C])   # [w, (h,b), out_ch]

    with tile.TileContext(nc) as tc:
        with (
            tc.tile_pool(name="bias", bufs=3) as biasp,
            tc.tile_pool(name="whh", bufs=4) as whhp,
            tc.tile_pool(name="cwt", bufs=1) as cwtp,
            tc.tile_pool(name="wstr", bufs=3) as wstrp,
            tc.tile_pool(name="x2t", bufs=1) as x2tp,
            tc.tile_pool(name="xin", bufs=1) as xinp,
            tc.tile_pool(name="usb", bufs=1) as usbp,
            tc.tile_pool(name="ut", bufs=3) as utp,
            tc.tile_pool(name="h2big", bufs=7) as h2bigp,
            tc.tile_pool(name="h1big", bufs=2) as h1bigp,
            tc.tile_pool(name="gw", bufs=1) as gwp,
            tc.tile_pool(name="cst", bufs=2) as cstp,
            tc.tile_pool(name="hsl", bufs=1) as hslp,
            tc.tile_pool(name="psu", bufs=2, space="PSUM") as psup,
            tc.tile_pool(name="psg", bufs=4, space="PSUM") as psgp,
            tc.tile_pool(name="dram", bufs=3, space="DRAM") as dramp,
        ):
            def load_bias(src, l, name):
                t_ = biasp.tile([32, GL], F32, tag="bias", name=name)
                nc.sync.dma_start(t_[:], src[l])
                return t_

            def load_w(w_dram, l, name, eng=None):
                # resident weight matrix -> [128, NT*GL] layout [p, kt*GL+n]
                wt = whhp.tile([128, NT * GL], BF16, tag="whh", name=name)
                (eng or nc.scalar).dma_start(
                    wt[:].rearrange("p (kt n) -> p kt n", kt=NT),
                    w_dram[l].rearrange("(kt p) n -> p kt n", p=128),
                )
                return wt

            def ag(ht):
                """AllGather this core's [256, 32] h slice -> [2048, 32].
                ht: [32, SL] block-transposed (block q col b row j =
                h[b, 32q+j]); cin[32q+j, b] = ht[j, 32q+b]."""
                cin = dramp.tile([2 * 128, NB], BF16, tag="agin")
                nc.sync.dma_start(
                    cin.rearrange("(q j) b -> j q b", j=32),
                    ht[:].rearrange("j (q b) -> j q b", b=NB))
                cout = dramp.tile([L, NB], BF16, tag="agout",
                                  addr_space="Shared")
                nc.gpsimd.collective_compute(
                    "AllGather", mybir.AluOpType.bypass,
                    replica_groups=[list(range(NCORES))],
                    ins=[cin[:]], outs=[cout[:]],
                )
                return cout

            def big_from_ag(cout, pool, tag):
                # SBUF bigtile [128, NT*32] layout [p, kt*32 + b]
                bt = pool.tile([128, NT * NB], BF16, tag=tag)
                nc.sync.dma_start(
                    bt[:].rearrange("p (kt b) -> p kt b", kt=NT),
                    cout.rearrange("(kt p) b -> p kt b", p=128),
                )
                return bt

            def x2t_from_ag(cout, x2t, t):
                # write h1T of step t into X2T columns kt*128 + t*32
                nc.sync.dma_start(
                    x2t[:].rearrange("p (kt t b) -> p kt t b",
                                     kt=NT, t=4)[:, :, t, :],
                    cout.rearrange("(kt p) b -> p kt b", p=128),
                )

            SIG = mybir.ActivationFunctionType.Sigmoid
            TANH = mybir.ActivationFunctionType.Tanh

            def cell(gsrc, add_ap, c_old, ltag, add2_ap=None, out_idx=None):
                """LSTM cell elementwise, partition base 0.
                gsrc: [psg0, psg1] PSUM pair or single [32, GL] AP.
                Returns (c_new, ht) with ht the bf16 block-transposed
                [32, SL] h slice ready for AllGather."""
                if isinstance(gsrc, (list, tuple)):
                    halves = [gsrc[0][:], gsrc[1][:]]
                else:
                    halves = [gsrc[:, 0:512], gsrc[:, 512:GL]]
                if add_ap is not None:
                    # separate half-tiles so ACT on half0 doesn't wait the
                    # DVE add of half1 (Tile deps are per-tile)
                    ga = gwp.tile([32, 512], F32, tag="ga")
                    gb = gwp.tile([32, 512], F32, tag="gb")
                    nc.vector.tensor_add(ga[:], halves[0], add_ap[:, 0:512])
                    nc.vector.tensor_add(gb[:], halves[1], add_ap[:, 512:GL])
                    if add2_ap is not None:
                        nc.vector.tensor_add(ga[:], ga[:], add2_ap[:, 0:512])
                        nc.vector.tensor_add(gb[:], gb[:], add2_ap[:, 512:GL])
                    halves = [ga[:], gb[:]]
                act = nc.scalar.activation
                if_t = gwp.tile([32, 2 * SL], F32, tag="ift")
                gt_t = gwp.tile([32, SL], F32, tag="gtt")
                o_t = gwp.tile([32, SL], F32, tag="ot")
                i_s = if_t[:, 0:SL]
                f_s = if_t[:, SL:2 * SL]
                act(if_t[:], halves[0], SIG)           # i, f fused
                act(gt_t[:], halves[1][:, 0:SL], TANH)
                act(o_t[:], halves[1][:, SL:2 * SL], SIG)
                c_new = cstp.tile([32, SL], F32, tag="c" + ltag)
                tmp = gwp.tile([32, SL], F32, tag="tmp")
                nc.vector.tensor_mul(tmp[:], i_s, gt_t[:])
                if c_old is not None:
                    cmul = gwp.tile([32, SL], F32, tag="cmul")
                    nc.vector.tensor_mul(cmul[:], f_s, c_old[:])
                    nc.vector.tensor_add(c_new[:], cmul[:], tmp[:])
                else:
                    nc.vector.tensor_copy(c_new[:], tmp[:])
                tanh_c = gwp.tile([32, SL], F32, tag="tanhc")
                act(tanh_c[:], c_new[:], TANH)
                hb = gwp.tile([32, SL], BF16, tag="hb")
                if out_idx is not None:
                    nc.vector.tensor_mul(tmp[:], o_t[:], tanh_c[:])
                    nc.sync.dma_start(chunks_out[out_idx], tmp[:])
                    nc.vector.tensor_copy(hb[:], tmp[:])
                else:
                    nc.vector.tensor_mul(hb[:], o_t[:], tanh_c[:])
                ht = hslp.tile([32, SL], BF16, tag="hsl")
                nc.vector.transpose(ht[:], hb[:])
                return c_new, ht

            def bulk_u(lhs_fn, w_dram, l, bias_ap):
                """U[t] = X[t] @ Wih_l^T for 4 steps; uts[1..3] are base-0
                [32, GL] fp32 tiles incl. bias; uts[0] None (u_sb[0:32])."""
                psums = [psup.tile([128, 512], F32, tag="psu",
                                   name=f"psu{n_}") for n_ in range(2)]
                for kt in range(NT):
                    wt = wstrp.tile([128, GL], BF16, tag="wstr")
                    nc.scalar.dma_start(
                        wt[:], w_dram[l, kt * 128:(kt + 1) * 128, :])
                    lhs = lhs_fn(kt)
                    for n in range(2):
                        nc.tensor.matmul(
                            psums[n][:], lhs, wt[:, n * 512:(n + 1) * 512],
                            start=(kt == 0), stop=(kt == NT - 1),
                            skip_group_check=True)
                u_sb = usbp.tile([128, GL], F32, tag="usb")
                for n in range(2):
                    nc.vector.tensor_copy(u_sb[:, n * 512:(n + 1) * 512],
                                          psums[n][:])
                uts = [None] * 4
                for t in range(1, 4):
                    ut = utp.tile([32, GL], F32, tag="ut")
                    nc.sync.dma_start(ut[:], u_sb[32 * t:32 * t + 32, :])
                    nc.vector.tensor_add(ut[:], ut[:], bias_ap)
                    uts[t] = ut
                return u_sb, uts

            def whh_matmuls(h_lhs_fn, whh_sb, extra=None):
                """K=2048 accumulation vs resident weights -> [psg0, psg1].
                extra: (lhs_fn, wih_sb) second K=2048 accumulation."""
                psums = [psgp.tile([32, 512], F32, tag="psg",
                                   name=f"psg{n_}") for n_ in range(2)]
                for kt in range(NT):
                    lhs = h_lhs_fn(kt)
                    for n in range(2):
                        nc.tensor.matmul(
                            psums[n][:], lhs,
                            whh_sb[:, kt * GL + n * 512:
                                   kt * GL + n * 512 + 512],
                            start=(kt == 0),
                            stop=(extra is None and kt == NT - 1),
                            skip_group_check=True)
                if extra is not None:
                    lhs2, wih_sb = extra
                    for kt in range(NT):
                        lhs = lhs2(kt)
                        for n in range(2):
                            nc.tensor.matmul(
                                psums[n][:], lhs,
                                wih_sb[:, kt * GL + n * 512:
                                       kt * GL + n * 512 + 512],
                                start=False, stop=(kt == NT - 1),
                                skip_group_check=True)
                return psums

            def load_xin(x_dram, name):
                # host pre-laid-out [p, kt*128 + t*32 + b]; one linear DMA
                t_ = xinp.tile([128, NT * 128], BF16, tag="xin", name=name)
                nc.sync.dma_start(t_[:], x_dram[:, :])
                return t_

            def x2t_block(x2t, kt, t):
                return x2t[:, kt * 128 + 32 * t: kt * 128 + 32 * t + 32]

            def wih_matmuls(lhs_fn, wih_sb):
                """K=2048 accumulation vs resident Wih only (layer-1 t0)."""
                psums = [psgp.tile([32, 512], F32, tag="psg",
                                   name=f"psgw{n_}") for n_ in range(2)]
                for kt in range(NT):
                    lhs = lhs_fn(kt)
                    for n in range(2):
                        nc.tensor.matmul(
                            psums[n][:], lhs,
                            wih_sb[:, kt * GL + n * 512:
                                   kt * GL + n * 512 + 512],
                            start=(kt == 0), stop=(kt == NT - 1),
                            skip_group_check=True)
                return psums

            def dual_scan(l0_init_lhs, uts, u_sb, whh0, b0, whh1, wih1, b1,
                          c1_init, c2_init, x2t_out, h2_init_big,
                          zero_init, store_de):
                """Wavefront over both layers: layer-0 (bulk-U + Whh0) and
                layer-1 (step-wise: Wih1 @ h1_t + Whh1 @ h2_{t-1}).
                Layer-1's matmuls fill layer-0's AllGather windows."""
                c1p, c2p = c1_init, c2_init
                h2_prev = h2_init_big
                for t in range(4):
                    # ---- layer 0 step t ----
                    if zero_init and t == 0:
                        c1p, ht = cell(u_sb[0:32, :], b0, None, "1")
                    else:
                        if t == 0:
                            lhs = l0_init_lhs
                        else:
                            lhs = lambda kt: x2t_block(x2t_out, kt, t - 1)
                        psums = whh_matmuls(lhs, whh0)
                        if uts[t] is not None:
                            c1p, ht = cell(psums, uts[t][:], c1p, "1")
                        else:
                            c1p, ht = cell(psums, u_sb[0:32, :], c1p, "1",
                                           add2_ap=b0)
                    cout = ag(ht)
                    x2t_from_ag(cout, x2t_out, t)
                    # ---- layer 1 step t ----
                    wih_lhs = lambda kt: x2t_block(x2t_out, kt, t)
                    if zero_init and t == 0:
                        psums = wih_matmuls(wih_lhs, wih1)
                        c2p, ht = cell(psums, b1, None, "2")
                    else:
                        h2b = h2_prev
                        psums = whh_matmuls(
                            lambda kt: h2b[:, kt * NB:kt * NB + NB], whh1,
                            extra=(wih_lhs, wih1))
                        c2p, ht = cell(
                            psums, b1, c2p, "2",
                            out_idx=(0 if store_de and t == 3 else None))
                    cout = ag(ht)
                    h2_prev = big_from_ag(cout, h2bigp, "h2big")
                return c1p, c2p, h2_prev

            # =========================================================
            # Phase E: encoder (batch 32 = [x2 fwd chain, x1-rev chain])
            # =========================================================
            eb0 = load_bias(eB, 0, "eb0")
            eb1 = load_bias(eB, 1, "eb1")
            whh_e0 = load_w(eWhh, 0, "whh_e0", eng=nc.sync)
            xe_sb = load_xin(xET, "xe_sb")
            u_sb, uts = bulk_u(
                lambda kt: xe_sb[:, kt * 128:(kt + 1) * 128], eWih, 0, eb0[:])
            whh_e1 = load_w(eWhh, 1, "whh_e1")
            wih_e1 = load_w(eWih, 1, "wih_e1")
            x2t_e = x2tp.tile([128, NT * 128], BF16, tag="x2t")
            c_e1, c_e2, h2_big = dual_scan(
                None, uts, u_sb, whh_e0, eb0[:], whh_e1, wih_e1, eb1[:],
                None, None, x2t_e, None, True, False)

            # =========================================================
            # Phase D1: decoder consume (batch = [x1 fwd, x2-rev])
            # =========================================================
            db0 = load_bias(dB, 0, "db0")
            db1 = load_bias(dB, 1, "db1")
            whh_d0 = load_w(dWhh, 0, "whh_d0")
            xd_sb = load_xin(xDT, "xd_sb")
            u_sb, uts = bulk_u(
                lambda kt: xd_sb[:, kt * 128:(kt + 1) * 128], dWih, 0, db0[:])
            whh_d1 = load_w(dWhh, 1, "whh_d1")
            wih_d1 = load_w(dWih, 1, "wih_d1")
            wih_d0 = load_w(dWih, 0, "wih_d0")
            x2t_d = x2tp.tile([128, NT * 128], BF16, tag="x2t")
            c1, c2, h2_big = dual_scan(
                lambda kt: x2t_block(x2t_e, kt, 3), uts, u_sb,
                whh_d0, db0[:], whh_d1, wih_d1, db1[:],
                c_e1, c_e2, x2t_d, h2_big, False, True)

            def emit_conv(cwt_sb, cb_sb):
                b1 = [conv_tiles["de"], conv_tiles["ar0"],
                      conv_tiles["ar1"], conv_tiles["ar2"]]
                b2 = [conv_tiles["ar2"], conv_tiles["ar1"],
                      conv_tiles["ar0"], conv_tiles["de"]]
                for w in range(4):
                    pcv = psup.tile([128, 512], F32, tag="psu",
                                    name=f"pcv{w}")
                    first = True
                    for br, src in ((0, b1[w]), (1, b2[w])):
                        lhs = src[:].rearrange("p (kt b) -> p kt b", kt=NT)
                        for j in range(4):
                            st = gwp.tile([128, 64], BF16, tag="cvl", bufs=2,
                                          name=f"cvl{w}_{br}_{j}")
                            nc.vector.tensor_copy(
                                st[:].rearrange("p (h b) -> p h b", h=4),
                                lhs[:, j::4, 16 * br:16 * br + 16])
                            nc.tensor.matmul(
                                pcv[0:64, :], st[:],
                                cwt_sb[:, (4 * br + j) * C:
                                       (4 * br + j + 1) * C],
                                start=first, stop=(br == 1 and j == 3))
                            first = False
                    cvs = gwp.tile([64, C], F32, tag="g", name=f"cvs{w}")
                    nc.vector.tensor_add(cvs[:], pcv[0:64, :], cb_sb[:])
                    cvo = gwp.tile([64, C], F32, tag="g2", name=f"cvo{w}")
                    nc.vector.tensor_scalar_mul(cvo[:], cvs[:], 0.2)
                    nc.vector.tensor_max(cvo[:], cvo[:], cvs[:])
                    nc.sync.dma_start(convout[w], cvo[:])

            # =========================================================
            # Phase D2: autoregressive decoder (7 steps, zero streaming)
            # =========================================================
            conv_tiles = {"de": h2_big}
            h1_big = None
            cb_sb = biasp.tile([64, C], F32, tag="bias", name="cb_sb")
            nc.sync.dma_start(cb_sb[:], cB[:])
            cwt_sb = cwtp.tile([128, 8 * C], BF16, tag="cwt")
            nc.scalar.dma_start(
                cwt_sb[:].rearrange("p (j o) -> p j o", j=8),
                cWT.rearrange("(j p) o -> p j o", p=128))

            for t in range(NAR):
                if t == 3:
                    emit_conv(cwt_sb, cb_sb)
                h2b, h1b = h2_big, h1_big
                if t == 0:
                    l0_lhs = lambda kt: x2t_block(x2t_d, kt, 3)
                else:
                    l0_lhs = lambda kt: h1b[:, kt * NB:kt * NB + NB]
                psums = whh_matmuls(
                    l0_lhs, whh_d0,
                    extra=(lambda kt: h2b[:, kt * NB:kt * NB + NB], wih_d0))
                c1, ht = cell(psums, db0[:], c1, "1")
                cout = ag(ht)
                h1_big = big_from_ag(cout, h1bigp, "h1big")

                h1b2 = h1_big
                psums = whh_matmuls(
                    lambda kt: h2b[:, kt * NB:kt * NB + NB], whh_d1,
                    extra=(lambda kt: h1b2[:, kt * NB:kt * NB + NB], wih_d1))
                c2, ht = cell(psums, db1[:], c2, "2", out_idx=t + 1)
                if t < NAR - 1:
                    cout = ag(ht)
                    h2_big = big_from_ag(cout, h2bigp, "h2big")
                    if t < 3:
                        conv_tiles[f"ar{t}"] = h2_big

    nc.compile()
    return nc


def _prep_inputs(x1, x2, enc_Wih, enc_Whh, enc_bih, enc_bhh,
                 dec_Wih, dec_Whh, dec_bih, dec_bhh, conv_W, conv_b):
    def colvecs(x):
        return [np.ascontiguousarray(x[:, :, :, t].reshape(B, L))
                for t in range(4)]

    x1c, x2c = colvecs(x1), colvecs(x2)

    def ximg(xa):
        # [4, L, NB] -> SBUF image [128, kt*128 + t*32 + b]
        return np.ascontiguousarray(
            xa.reshape(4, NT, 128, NB).transpose(2, 1, 0, 3)
            .reshape(128, NT * 4 * NB)).astype(NPBF)

    xET = ximg(np.stack([
        np.concatenate([x2c[t], x1c[3 - t]], axis=0)[:, PERM].T
        for t in range(4)]))
    xDT = ximg(np.stack([
        np.concatenate([x1c[t], x2c[3 - t]], axis=0)[:, PERM].T
        for t in range(4)]))

    def prep_core(k, Wih, Whh, bih, bhh):
        rows = np.concatenate([g * L + PERM[k * SL:(k + 1) * SL]
                               for g in range(4)])
        wihT = np.stack([np.ascontiguousarray(Wih[l][rows][:, PERM].T)
                         for l in range(2)])
        whhT = np.stack([np.ascontiguousarray(Whh[l][rows][:, PERM].T)
                         for l in range(2)])
        bb = np.stack([(bih[l] + bhh[l])[rows] for l in range(2)])
        brep = np.broadcast_to(bb[:, None, :], (2, 32, GL)).copy()
        return wihT.astype(NPBF), whhT.astype(NPBF), brep.astype(np.float32)

    cWT = np.ascontiguousarray(conv_W.T).astype(NPBF)
    cBr = np.broadcast_to(conv_b[None, :], (64, C)).copy().astype(np.float32)

    in_maps = []
    for k in range(NCORES):
        eWihT, eWhhT, eBr = prep_core(k, enc_Wih, enc_Whh, enc_bih, enc_bhh)
        dWihT, dWhhT, dBr = prep_core(k, dec_Wih, dec_Whh, dec_bih, dec_bhh)
        in_maps.append({
            "xET": xET, "xDT": xDT,
            "eWih": eWihT, "eWhh": eWhhT, "eB": eBr,
            "dWih": dWihT, "dWhh": dWhhT, "dB": dBr,
            "cWT": cWT, "cB": cBr,
        })
    return in_maps


def _postprocess(results, x1, x2):
    # gather chunk slices across cores: core k owns features [256k:256k+256)
    chunks = np.zeros((8, B * 2, L), np.float32)
    for k in range(NCORES):
        chunks[:, :, k * SL:(k + 1) * SL] = results[k]["chunks_out"]
    convout = results[0]["convout"]

    def tochunk(t, half):
        v = chunks[t, half * B:(half + 1) * B, :]   # [16, L] dev order
        return v[:, IPERM].reshape(B, C, H)

    de1 = tochunk(0, 0)
    p1 = [tochunk(1 + j, 0) for j in range(NAR)]
    de2 = tochunk(0, 1)
    p2 = [tochunk(1 + j, 1) for j in range(NAR)]

    mid1 = np.stack([de1, p1[0], p1[1], p1[2]], axis=-1)
    tail1 = np.stack([p1[3], p1[4], p1[5], p1[6]], axis=-1)
    head2 = np.stack([p2[6], p2[5], p2[4], p2[3]], axis=-1)
    mid2 = np.stack([p2[2], p2[1], p2[0], de2], axis=-1)

    out = convout.reshape(4, 4, B, C).transpose(2, 3, 1, 0)
    out = np.ascontiguousarray(out, dtype=np.float32)
    return (out, np.asarray(x1), mid1, tail1, head2, mid2, np.asarray(x2))


def _run(in_maps, trace=False):
    if "nc" not in _CACHE:
        _CACHE["nc"] = _build_nc()
        _CACHE["tmpdir"] = tempfile.mkdtemp(prefix="lstmk_")
    nc = _CACHE["nc"]
    res = bass_utils.run_bass_kernel_spmd(
        nc, in_maps, core_ids=list(range(NCORES)), trace=trace,
        tmpdir=_CACHE["tmpdir"] if trace else None)
    return res


def kernel(**inputs):
    inputs = {k: np.asarray(v, dtype=np.float32) for k, v in inputs.items()}
    in_maps = _prep_inputs(**inputs)
    res = _run(in_maps, trace=False)
    return _postprocess(res.results, inputs["x1"], inputs["x2"])


def kernel_traced(**inputs):
    inputs = {k: np.asarray(v, dtype=np.float32) for k, v in inputs.items()}
    in_maps = _prep_inputs(**inputs)
    res = _run(in_maps, trace=True)
    return _postprocess(res.results, inputs["x1"], inputs["x2"]), res

